# revision 75
# baseline (speedup 1.0000x reference)
"""Trainium2 Bass kernel for nn_Attention_27376121544790.

Math (per batch element, B=8 -> one element per NeuronCore, no collectives):
  qk   = x @ W.T + b                              [N, D] (on device: [D, N])
  q = k = l2norm(qk per 64-dim head)
  S    = (q @ k.T) * (sqrt(64)/attn_gamma)        per head
  attn = softmax(S) = E / Z,  E = exp(S), Z = col sums (E symmetric)
  out  = attn @ v,  v = x head-split
  final= w0*(out @ W.T) + w1*qk + (1-w1)*b        (uses x@W.T = qk - b, so the
         blend projection collapses into the already-computed qk)

Key engine assignments / formats (ACT exp of the N^2 logits is the
bottleneck engine; everything else is arranged to keep it fed):
  - proj1 (x@W.T) in bf16 (accuracy: qk feeds the final output directly),
    W.T and x.T interleaved in one "wx" tensor so the contraction loop can
    chase combined chunk-pair DMAs.
  - q/k (qn8), E, v (xaug), att and W8 in fp8e4 DoubleRow pair layouts
    ([Ki, 2, free]); the gram, attn@v and final att-projection all run as
    fp8 DoubleRow matmuls. att is scaled x32 into fp8 range via a 1/32
    ones column in xaug (Z comes out as Z/32; recip gives 32/Z); W8 is
    32*W; qn8 is 8*qn (compensated in the exp scale ls/64).
  - l2norm: per-chunk ssq via block-ones matmul; invn = rsqrt(ssq) via the
    quake bit-trick + one Newton step on DVE (SBUF staging first: PSUM
    reads convert, they do not reinterpret bits). Chunk 0 instead uses
    ACT Ln/Exp while ACT is still idle. Broadcast across partitions on
    gpsimd (which can never touch PSUM).
  - addend w1*qk+(1-w1)*b is pre-scaled by 1/w0s and folded into the
    proj2 accumulator through an identity matmul, so proj2 drains are
    pure scale-copies alternating ACT/DVE.
  - PE p-state: a dependency-free warmup block ramps the tensor engine
    to full clock before the first real matmul; the ramp persists.
  - Window pipeline: proj1 two chunks ahead, norm chain one ahead; heads
    (2c, 2c+1) start as soon as chunk c is normalized, overlapping the
    projection and norm work with the ACT-bound softmax stream.
"""

import math
import os

import numpy as np

B, N, C, D = 8, 1024, 1024, 1024
HEADS, HD = 16, 64
P = 128
EPS = 1e-6
NCHUNK = C // P      # 8 chunks of 128 feature rows
NPAIR = NCHUNK // 2  # 4 DoubleRow pair chunks
FH = 512             # free-dim half (one PSUM bank of f32)
HP = 80              # padded per-head stride in xaug (65 used, %16 == 0)
W8S = 32.0           # fp8 scale on W
VSC = 32.0           # att scale (via 1/32 ones column)


def _build(gamma: float, w0: float, w1: float, logit_scale: float):
    import concourse.bass as bass
    import concourse.tile as tile
    from concourse import bacc, mybir

    f32 = mybir.dt.float32
    f32r = mybir.dt.float32r
    BF16 = mybir.dt.bfloat16
    FP8 = mybir.dt.float8e4
    DR = mybir.MatmulPerfMode.DoubleRow

    Exp = mybir.ActivationFunctionType.Exp
    MULT = mybir.AluOpType.mult
    ADD = mybir.AluOpType.add
    SUB = mybir.AluOpType.subtract
    LSR = mybir.AluOpType.logical_shift_right
    i32 = mybir.dt.int32
    QC = 0x5F3759DF  # quake rsqrt seed constant

    W0S = w0 / (W8S * VSC)  # proj2 drain scale

    nc = bacc.Bacc("TRN2", target_bir_lowering=False, debug=False)

    wx_d = nc.declare_dram_parameter("wx", [C, 2, N], BF16, isOutput=False)
    if gamma != 0.0:
        xT0_d = nc.declare_dram_parameter("xT0", [C, N], BF16, isOutput=False)
    W8_d = nc.declare_dram_parameter("W8", [NPAIR * P, 2, D], FP8, isOutput=False)
    xa_d = nc.declare_dram_parameter("xa", [NPAIR * P, 2, HEADS * HP], FP8, isOutput=False)
    bdc_d = nc.declare_dram_parameter("bdc", [P, 2], BF16, isOutput=False)
    bd2_d = nc.declare_dram_parameter("bd2", [2, P], BF16, isOutput=False)
    id_d = nc.declare_dram_parameter("idn", [P, P], f32r, isOutput=False)
    bmat_d = nc.declare_dram_parameter("bmat", [P, NCHUNK], f32, isOutput=False)
    bmat2_d = nc.declare_dram_parameter("bmat2", [P, NCHUNK], f32, isOutput=False)
    out_d = nc.declare_dram_parameter("out", [D, N], BF16, isOutput=True)

    with tile.TileContext(nc) as tc:
        with (
            tc.tile_pool(name="pers", bufs=1) as pers,
            tc.tile_pool(name="small", bufs=1) as small,
            tc.tile_pool(name="sqp", bufs=2) as sqp,
            tc.tile_pool(name="invp", bufs=2) as invp,
            tc.tile_pool(name="qnp", bufs=2) as qnp,
            tc.tile_pool(name="fin", bufs=3) as pfin,
            tc.tile_pool(name="psum_p1", bufs=2, space="PSUM") as pp1,
        ):
            # ---- persistent SBUF ----
            wx_t = [pers.tile([P, 2, N], BF16, tag=f"wx{c}", name=f"wx{c}") for c in range(NCHUNK)]
            W8_t = [pers.tile([P, 2, D], FP8, tag=f"W8{p}", name=f"W8{p}") for p in range(NPAIR)]
            xa_t = [pers.tile([P, 2, HEADS * HP], FP8, tag=f"xa{p}", name=f"xa{p}") for p in range(NPAIR)]
            qkT_t = [pers.tile([P, N], f32r, tag=f"qk{c}", name=f"qk{c}") for c in range(NCHUNK)]
            qn8_t = [pers.tile([32, 2, 2, N], FP8, tag=f"q8{c}", name=f"q8{c}") for c in range(NCHUNK)]
            att_t = [pers.tile([P, 2, N], FP8, tag=f"at{p}", name=f"at{p}") for p in range(NPAIR)]
            if gamma != 0.0:
                xT0_t = [pers.tile([P, N], BF16, tag=f"x0{c}", name=f"x0{c}") for c in range(NCHUNK)]
                ad_t = [pers.tile([P, N], f32r, tag=f"ad{c}", name=f"ad{c}") for c in range(NCHUNK)]
            else:
                ad_t = qkT_t

            bdc_t = small.tile([P, 2], BF16, tag="bdc")
            bd2_t = small.tile([2, P], BF16, tag="bd2")
            bmat_t = small.tile([P, NCHUNK], f32, tag="bmat")
            bmat2_t = small.tile([P, NCHUNK], f32, tag="bmat2")
            id_t = small.tile([P, P], f32r, tag="idn")

            # PE p-state warmup: ~5us of dependency-free matmuls during the
            # input-DMA wait ramps the tensor engine to full clock; the ramp
            # state persists across later idle gaps.
            warm_t = small.tile([P, FH], BF16, tag="warm")
            nc.gpsimd.memset(warm_t[:], 0.25)
            wps = pp1.tile([P, FH], f32, tag="p1", name="wps")
            for i in range(8):
                nc.tensor.matmul(wps[:], warm_t[:, 0:P], warm_t[:],
                                 start=True, stop=True)

            nc.sync.dma_start(bdc_t[:], bdc_d[:])
            nc.sync.dma_start(bmat_t[:], bmat_d[:])
            # combined W.T|x.T chunk loads: proj1(0)'s k-th matmul needs the
            # k-th chunk of both tensors; one DMA delivers the pair
            for c in range(NCHUNK):
                nc.sync.dma_start(wx_t[c][:], wx_d[c * P:(c + 1) * P, :, :])
            nc.sync.dma_start(bd2_t[:], bd2_d[:])
            for p in range(NPAIR):
                nc.sync.dma_start(W8_t[p][:], W8_d[p * P:(p + 1) * P, :, :])
                nc.sync.dma_start(xa_t[p][:], xa_d[p * P:(p + 1) * P, :, :])
            nc.sync.dma_start(bmat2_t[:], bmat2_d[:])
            nc.sync.dma_start(id_t[:], id_d[:])
            if gamma != 0.0:
                for c in range(NCHUNK):
                    nc.sync.dma_start(xT0_t[c][:], xT0_d[c * P:(c + 1) * P, :])

            def emit_proj1(c):
                # qk_c = (x @ W.T)_c + b_c
                for fn in range(2):
                    ps = pp1.tile([P, FH], f32, tag="p1", name="ps")
                    for k in range(NCHUNK):
                        nc.tensor.matmul(
                            ps[:],
                            wx_t[k][:, 0, c * P:(c + 1) * P],
                            wx_t[k][:, 1, fn * FH:(fn + 1) * FH],
                            start=(k == 0), stop=(k == NCHUNK - 1))
                    nc.vector.tensor_scalar_add(
                        qkT_t[c][:, fn * FH:(fn + 1) * FH], ps[:],
                        bmat_t[:, c:c + 1])
                if gamma != 0.0:
                    # separate x @ W.T for the final-output addend
                    for fn in range(2):
                        ps = pp1.tile([P, FH], f32, tag="p1", name="ps0")
                        for k in range(NCHUNK):
                            nc.tensor.matmul(
                                ps[:],
                                wx_t[k][:, 0, c * P:(c + 1) * P],
                                xT0_t[k][:, fn * FH:(fn + 1) * FH],
                                start=(k == 0), stop=(k == NCHUNK - 1))
                        nc.vector.tensor_scalar(
                            ad_t[c][:, fn * FH:(fn + 1) * FH], ps[:],
                            float(w1 / W0S), bmat2_t[:, c:c + 1], MULT, ADD)

            qn_stage = {}

            def emit_norm(c, psum_pool=None):
                # l2norm pair for heads (2c, 2c+1): invn = rsqrt(ssq) via
                # quake seed + one Newton step (DVE; Pool only does sq so the
                # Pool FIFO stays clear for the per-head Z broadcasts).
                # chunk 0 borrows the (still idle) gram psum pool and runs sq
                # on DVE so the startup chain never blocks proj1(1+) slots.
                pool = psum_pool if psum_pool is not None else pp1
                sq = sqp.tile([P, N], BF16, tag="sq", name="sq")
                if c == 0:
                    nc.vector.tensor_mul(sq[:], qkT_t[c][:], qkT_t[c][:])
                else:
                    nc.gpsimd.tensor_mul(sq[:], qkT_t[c][:], qkT_t[c][:])
                y0 = invp.tile([2, N], i32, tag="y0", name="y0")
                sd = invp.tile([2, N], f32, tag="sd", name="sd")
                t2 = invp.tile([2, N], f32, tag="t2", name="t2")
                inv = invp.tile([2, N], BF16, tag="inv", name="inv")
                ssqs = []
                for fn in range(2):
                    ssq = pool.tile([P, FH] if psum_pool is None else [P, N],
                                    f32, tag="p1" if psum_pool is None else "pg",
                                    name="ssq")
                    nc.tensor.matmul(
                        ssq[0:2, 0:FH], bdc_t[:], sq[:, fn * FH:(fn + 1) * FH],
                        start=True, stop=True)
                    ssqs.append(ssq)
                if c == 0:
                    # startup chunk: ACT is idle, so do rsqrt as
                    # exp(-0.5*ln(ssq)) there; both table loads hide in the
                    # pre-attention idle and the DVE chain shortens by ~8us
                    Ln = mybir.ActivationFunctionType.Ln
                    for fn in range(2):
                        nc.scalar.activation(
                            sd[:, fn * FH:(fn + 1) * FH], ssqs[fn][0:2, 0:FH],
                            Ln)
                    nc.scalar.activation(inv[:], sd[:], Exp, scale=-0.5)
                else:
                    for fn in range(2):
                        # PSUM reads convert (not reinterpret), so stage ssq
                        # in SBUF before the integer bit-trick reads below.
                        nc.vector.tensor_copy(
                            sd[:, fn * FH:(fn + 1) * FH], ssqs[fn][0:2, 0:FH])
                    # quake seed: y0 = QC - (ssq_bits >> 1), one Newton step
                    nc.vector.tensor_scalar(y0[:], sd[:].bitcast(i32), 1,
                                            None, LSR)
                    y0f = y0[:].bitcast(f32)
                    nc.vector.tensor_scalar(y0[:], y0[:], QC, -1, SUB, MULT)
                    nc.vector.tensor_mul(t2[:], y0f, y0f)
                    nc.vector.tensor_mul(sd[:], sd[:], t2[:])
                    # inv = y0 * (1.5 - 0.5*x*y0^2)
                    nc.vector.tensor_scalar(t2[:], sd[:], -0.5, 1.5, MULT, ADD)
                    nc.vector.tensor_mul(inv[:], t2[:], y0f)

                # qn = qk * bcast(8*invn) -> fp8 (bd2 holds 8.0s: fp8 scale)
                qn = qnp.tile([P, N], FP8, tag="qn", name="qn")
                for fn in range(2):
                    pbt = pool.tile([P, FH] if psum_pool is None else [P, N],
                                    f32, tag="p1" if psum_pool is None else "pg",
                                    name="pbt")
                    nc.tensor.matmul(
                        pbt[:, 0:FH], bd2_t[:], inv[:, fn * FH:(fn + 1) * FH],
                        start=True, stop=True)
                    nc.vector.tensor_mul(
                        qn[:, fn * FH:(fn + 1) * FH],
                        qkT_t[c][:, fn * FH:(fn + 1) * FH], pbt[:, 0:FH])
                qn_stage[c] = qn
                if c == 0:
                    return  # heads 0/1 read the staging tile directly
                # relayout each half into DoubleRow [32, 2, N] with one DMA:
                # flat AP pairing maps head-dim d = 2p+s, which is a valid
                # contraction order because the gram uses the same tile on
                # both sides
                for half in range(2):
                    nc.sync.dma_start(
                        qn8_t[c][:, half, :, :],
                        qn[half * HD:(half + 1) * HD, :])

            with (
                tc.tile_pool(name="E", bufs=7) as pE,
                tc.tile_pool(name="rzp", bufs=2) as rzp,
                tc.tile_pool(name="psum_g", bufs=2, space="PSUM") as pg_pool,
                tc.tile_pool(name="psum_av", bufs=1, space="PSUM") as pav_pool,
            ):
                def emit_head(h):
                    c, half = h // 2, h % 2
                    pav = pav_pool.tile([HD + 1, 2, FH], f32, tag="pav", name="pav")
                    for p in range(NPAIR):
                        Ep = pE.tile([P, 2, N], FP8, tag="E", name="Ep")
                        for s in range(2):
                            mb = 2 * p + s
                            pg = pg_pool.tile([P, N], f32, tag="pg", name="pg")
                            for fn in range(2):
                                if c == 0:
                                    # chunk 0: plain fp8 matmul off the qn
                                    # staging tile (no relayout wait)
                                    qs = qn_stage[0][half * HD:(half + 1) * HD, :]
                                    nc.tensor.matmul(
                                        pg[:, fn * FH:(fn + 1) * FH],
                                        qs[:, mb * P:(mb + 1) * P],
                                        qs[:, fn * FH:(fn + 1) * FH],
                                        start=True, stop=True)
                                else:
                                    qn_h = qn8_t[c][:, half, :, :]
                                    nc.tensor.matmul(
                                        pg[:, fn * FH:(fn + 1) * FH],
                                        qn_h[:, :, mb * P:(mb + 1) * P],
                                        qn_h[:, :, fn * FH:(fn + 1) * FH],
                                        start=True, stop=True, perf_mode=DR)
                            nc.scalar.activation(Ep[:, s, :], pg[:], Exp,
                                                 scale=logit_scale / 64.0)
                        for fn in range(2):
                            nc.tensor.matmul(
                                pav[:, fn, :],
                                xa_t[p][:, :, h * HP:h * HP + HD + 1],
                                Ep[:, :, fn * FH:(fn + 1) * FH],
                                start=(p == 0), stop=(p == NPAIR - 1),
                                perf_mode=DR)
                    # Z row must reach SBUF via a plain DVE copy: the custom
                    # reciprocal ucode (and all gpsimd ops) cannot read PSUM.
                    zr = rzp.tile([1, 2, FH], f32, tag="zr", name="zr")
                    nc.vector.tensor_copy(zr[:], pav[HD:HD + 1, :, :])
                    rz = rzp.tile([1, 2, FH], f32, tag="rz", name="rz")
                    nc.vector.reciprocal_approx_fast(rz[:], zr[:])
                    rzb = rzp.tile([P, N], f32, tag="rzb", name="rzb")
                    nc.gpsimd.partition_broadcast(rzb[:], rz[:])
                    ap, sp = h // 4, (h // 2) % 2
                    for fn in range(2):
                        nc.vector.tensor_mul(
                            att_t[ap][half * HD:(half + 1) * HD, sp,
                                      fn * FH:(fn + 1) * FH],
                            pav[0:HD, fn, :],
                            rzb[half * HD:(half + 1) * HD,
                                fn * FH:(fn + 1) * FH])

                # window pipeline: proj1 one chunk ahead; the norm chain for
                # chunk c+1 is emitted between the two heads of chunk c so
                # the Pool FIFO serves head (2c)'s broadcast before chunk
                # c+1's sq (avoids head-of-line blocking).
                def emit_addend(c):
                    # addend (prescaled by 1/w0s so the proj2 drain is a pure
                    # psum*w0s copy on ACT): ad = (w1*qk + (1-w1)*b) / w0s,
                    # in place over qk (Pool; emitted a window after its
                    # dependencies clear so it never stalls the Pool FIFO)
                    if gamma == 0.0:
                        nc.gpsimd.tensor_scalar(
                            qkT_t[c][:], qkT_t[c][:],
                            float(w1 / W0S), bmat2_t[:, c:c + 1], MULT, ADD)

                # software pipeline: proj1 two windows ahead, norm one ahead
                emit_proj1(0)
                emit_norm(0, pg_pool)
                emit_proj1(1)
                for c in range(NCHUNK):
                    if c + 2 < NCHUNK:
                        emit_proj1(c + 2)
                    emit_head(2 * c)
                    if c + 1 < NCHUNK:
                        emit_norm(c + 1)
                    if c >= 1:
                        emit_addend(c - 1)
                    emit_head(2 * c + 1)
                emit_addend(NCHUNK - 1)

                if os.environ.get("BK_DEBUG", "0") == "1":
                    dbg = {
                        "d_qk0": qkT_t[0],
                        "d_qn8_1": qn8_t[1],
                        "d_att0": att_t[0],
                        "d_xa0": xa_t[0],
                        "d_w80": W8_t[0],
                    }
                    for nm, t in dbg.items():
                        sh = list(t[:].shape)
                        flat = [sh[0], int(np.prod(sh[1:]))]
                        dd = nc.declare_dram_parameter(nm, flat, t.tensor.dtype,
                                                       isOutput=True)
                        nc.sync.dma_start(dd[:], t[:])

                # ---- final projection: fin = w0s*psum, psum = att@W8 + ad
                # (reuses the gram psum pool -- no pool-close barrier -- and
                # alternates odd groups onto the p1 pool so three psum groups
                # stay in flight; drains alternate ACT/DVE)
                for m in range(NCHUNK):
                    fin = pfin.tile([P, N], BF16, tag="fin", name="fin")
                    if m % 2 == 0:
                        ps2f = [pg_pool.tile([P, N], f32, tag="pg", name="ps2")]
                        slc = [(ps2f[0], fn * FH) for fn in range(2)]
                    else:
                        a = pp1.tile([P, FH], f32, tag="p1", name="p2a")
                        bb = pp1.tile([P, FH], f32, tag="p1", name="p2b")
                        slc = [(a, 0), (bb, 0)]
                    for fn in range(2):
                        t, off = slc[fn]
                        # pairs 0-2 and the addend identity are ready well
                        # before the last head; only the pair-3 matmul (and
                        # the group stop) waits for heads 12-15
                        for p in range(NPAIR - 1):
                            nc.tensor.matmul(
                                t[:, off:off + FH],
                                W8_t[p][:, :, m * P:(m + 1) * P],
                                att_t[p][:, :, fn * FH:(fn + 1) * FH],
                                start=(p == 0), stop=False,
                                perf_mode=DR)
                        nc.tensor.matmul(
                            t[:, off:off + FH],
                            id_t[:],
                            ad_t[m][:, fn * FH:(fn + 1) * FH],
                            start=False, stop=False)
                        nc.tensor.matmul(
                            t[:, off:off + FH],
                            W8_t[NPAIR - 1][:, :, m * P:(m + 1) * P],
                            att_t[NPAIR - 1][:, :, fn * FH:(fn + 1) * FH],
                            start=False, stop=True,
                            perf_mode=DR)
                        if m % 2 == 1:
                            if fn == 0:
                                nc.scalar.mul(
                                    fin[:, 0:FH], t[:, 0:FH], float(W0S))
                            else:
                                nc.vector.tensor_scalar_mul(
                                    fin[:, FH:N], t[:, 0:FH], float(W0S))
                    if m % 2 == 0:
                        nc.vector.tensor_scalar_mul(
                            fin[:], slc[0][0][:], float(W0S))
                    nc.sync.dma_start(out_d[m * P:(m + 1) * P, :], fin[:])

    nc.compile()
    return nc


def _host_prep(x, pos, W, b, gamma, w0, w1):
    """Per-core input shards (host layout work only)."""
    import ml_dtypes

    bf16 = ml_dtypes.bfloat16
    f8 = ml_dtypes.float8_e4m3

    WT = np.ascontiguousarray(W.T)                        # [C, D] f32
    WTb = WT.astype(bf16)                                 # [C, D] bf16
    # pair layout: W8[pair*128 + part, s, d] = 32*W.T[128*(2*pair+s)+part, d]
    W8 = (W8S * WT).astype(f8).reshape(NPAIR, 2, P, D).transpose(0, 2, 1, 3)
    W8 = np.ascontiguousarray(W8).reshape(NPAIR * P, 2, D)
    bmat = np.ascontiguousarray(b.reshape(NCHUNK, P).T)   # [P, 8]
    w0s = w0 / (W8S * VSC)
    bmat2 = np.ascontiguousarray(
        ((b / w0s if gamma != 0.0 else (1.0 - w1) / w0s * b)).reshape(NCHUNK, P).T)
    idn = np.eye(P, dtype=np.float32)
    bdc = np.zeros((P, 2), dtype=bf16)
    bdc[:HD, 0] = 1.0
    bdc[HD:, 1] = 1.0
    # 8.0 (exact in bf16): scales qn into fp8-friendly range; compensated by
    # exp(scale=logit_scale/64)
    bd2 = np.zeros((2, P), dtype=bf16)
    bd2[0, :HD] = 8.0
    bd2[1, HD:] = 8.0

    in_maps = []
    for i in range(B):
        xi = x[i]                                         # [N, C]
        xa = np.zeros((N, HEADS, HP), dtype=np.float32)
        xa[:, :, :HD] = xi.reshape(N, HEADS, HD)
        xa[:, :, HD] = 1.0 / VSC
        xa8 = xa.astype(f8).reshape(NPAIR, 2, P, HEADS * HP)
        xa8 = np.ascontiguousarray(xa8.transpose(0, 2, 1, 3))
        m = {
            "W8": W8,
            "xa": xa8.reshape(NPAIR * P, 2, HEADS * HP),
            "bdc": bdc,
            "bd2": bd2,
            "idn": idn,
            "bmat": bmat,
            "bmat2": bmat2,
        }
        wx = np.empty((C, 2, N), dtype=bf16)
        wx[:, 0, :] = WTb
        if gamma != 0.0:
            xp = xi + gamma * pos[i].reshape(C, N).T
            wx[:, 1, :] = xp.T.astype(bf16)
            m["xT0"] = np.ascontiguousarray(xi.T).astype(bf16)
        else:
            wx[:, 1, :] = xi.T.astype(bf16)
        m["wx"] = wx
        in_maps.append(m)
    return in_maps


LAST_RESULT = None


def kernel(x, pos, W, b, gamma, attn_gamma, sum_gamma0, sum_gamma1):
    global LAST_RESULT
    import sys
    sys.path.insert(0, "/opt/trn_rl_repo")
    from concourse.bass_utils import run_bass_kernel_spmd

    x = np.asarray(x, dtype=np.float32)
    pos = np.asarray(pos, dtype=np.float32)
    W = np.asarray(W, dtype=np.float32)
    b = np.asarray(b, dtype=np.float32)
    gamma = float(np.asarray(gamma))
    attn_gamma = float(np.asarray(attn_gamma))
    g0 = math.exp(float(np.asarray(sum_gamma0)))
    g1 = math.exp(float(np.asarray(sum_gamma1)))
    w0, w1 = g0 / (g0 + g1), g1 / (g0 + g1)
    logit_scale = math.sqrt(HD) / attn_gamma

    nc = _build(gamma, w0, w1, logit_scale)
    in_maps = _host_prep(x, pos, W, b, gamma, w0, w1)
    res = run_bass_kernel_spmd(
        nc, in_maps, core_ids=list(range(B)),
        trace=os.environ.get("BK_TRACE", "0") == "1",
    )
    LAST_RESULT = res
    out = np.empty((B, N, D), dtype=np.float32)
    for i in range(B):
        out[i] = res.results[i]["out"].astype(np.float32).T
    return out


# revision 78
# speedup vs baseline: 1.0054x; 1.0054x over previous
"""Trainium2 Bass kernel for nn_Attention_27376121544790.

Math (per batch element, B=8 -> one element per NeuronCore, no collectives):
  qk   = x @ W.T + b                              [N, D] (on device: [D, N])
  q = k = l2norm(qk per 64-dim head)
  S    = (q @ k.T) * (sqrt(64)/attn_gamma)        per head
  attn = softmax(S) = E / Z,  E = exp(S), Z = col sums (E symmetric)
  out  = attn @ v,  v = x head-split
  final= w0*(out @ W.T) + w1*qk + (1-w1)*b        (uses x@W.T = qk - b, so the
         blend projection collapses into the already-computed qk)

Key engine assignments / formats (ACT exp of the N^2 logits is the
bottleneck engine; everything else is arranged to keep it fed):
  - proj1 (x@W.T) in bf16 (accuracy: qk feeds the final output directly),
    W.T and x.T interleaved in one "wx" tensor so the contraction loop can
    chase combined chunk-pair DMAs.
  - q/k (qn8), E, v (xaug), att and W8 in fp8e4 DoubleRow pair layouts
    ([Ki, 2, free]); the gram, attn@v and final att-projection all run as
    fp8 DoubleRow matmuls. att is scaled x32 into fp8 range via a 1/32
    ones column in xaug (Z comes out as Z/32; recip gives 32/Z); W8 is
    32*W; qn8 is 8*qn (compensated in the exp scale ls/64).
  - l2norm: per-chunk ssq via block-ones matmul; invn = rsqrt(ssq) via the
    quake bit-trick + one Newton step on DVE (SBUF staging first: PSUM
    reads convert, they do not reinterpret bits). Chunk 0 instead uses
    ACT Ln/Exp while ACT is still idle. Broadcast across partitions on
    gpsimd (which can never touch PSUM).
  - addend w1*qk+(1-w1)*b is pre-scaled by 1/w0s and folded into the
    proj2 accumulator through an identity matmul, so proj2 drains are
    pure scale-copies alternating ACT/DVE.
  - PE p-state: a dependency-free warmup block ramps the tensor engine
    to full clock before the first real matmul; the ramp persists.
  - Window pipeline: proj1 two chunks ahead, norm chain one ahead; heads
    (2c, 2c+1) start as soon as chunk c is normalized, overlapping the
    projection and norm work with the ACT-bound softmax stream.
"""

import math
import os

import numpy as np

B, N, C, D = 8, 1024, 1024, 1024
HEADS, HD = 16, 64
P = 128
EPS = 1e-6
NCHUNK = C // P      # 8 chunks of 128 feature rows
NPAIR = NCHUNK // 2  # 4 DoubleRow pair chunks
FH = 512             # free-dim half (one PSUM bank of f32)
HP = 80              # padded per-head stride in xaug (65 used, %16 == 0)
W8S = 32.0           # fp8 scale on W
VSC = 32.0           # att scale (via 1/32 ones column)


def _build(gamma: float, w0: float, w1: float, logit_scale: float):
    import concourse.bass as bass
    import concourse.tile as tile
    from concourse import bacc, mybir

    f32 = mybir.dt.float32
    f32r = mybir.dt.float32r
    BF16 = mybir.dt.bfloat16
    FP8 = mybir.dt.float8e4
    DR = mybir.MatmulPerfMode.DoubleRow

    Exp = mybir.ActivationFunctionType.Exp
    MULT = mybir.AluOpType.mult
    ADD = mybir.AluOpType.add
    SUB = mybir.AluOpType.subtract
    LSR = mybir.AluOpType.logical_shift_right
    i32 = mybir.dt.int32
    QC = 0x5F3759DF  # quake rsqrt seed constant

    W0S = w0 / (W8S * VSC)  # proj2 drain scale

    nc = bacc.Bacc("TRN2", target_bir_lowering=False, debug=False)

    wx_d = nc.declare_dram_parameter("wx", [C, 2, N], BF16, isOutput=False)
    if gamma != 0.0:
        xT0_d = nc.declare_dram_parameter("xT0", [C, N], BF16, isOutput=False)
    W8_d = nc.declare_dram_parameter("W8", [NPAIR * P, 2, D], FP8, isOutput=False)
    xa_d = nc.declare_dram_parameter("xa", [NPAIR * P, 2, HEADS * HP], FP8, isOutput=False)
    bdc_d = nc.declare_dram_parameter("bdc", [P, 2], BF16, isOutput=False)
    bd2_d = nc.declare_dram_parameter("bd2", [2, P], BF16, isOutput=False)
    id_d = nc.declare_dram_parameter("idn", [P, P], f32r, isOutput=False)
    bmat_d = nc.declare_dram_parameter("bmat", [P, NCHUNK], f32, isOutput=False)
    bmat2_d = nc.declare_dram_parameter("bmat2", [P, NCHUNK], f32, isOutput=False)
    out_d = nc.declare_dram_parameter("out", [D, N], BF16, isOutput=True)

    with tile.TileContext(nc) as tc:
        with (
            tc.tile_pool(name="pers", bufs=1) as pers,
            tc.tile_pool(name="small", bufs=1) as small,
            tc.tile_pool(name="sqp", bufs=2) as sqp,
            tc.tile_pool(name="invp", bufs=2) as invp,
            tc.tile_pool(name="qnp", bufs=2) as qnp,
            tc.tile_pool(name="fin", bufs=3) as pfin,
            tc.tile_pool(name="psum_p1", bufs=2, space="PSUM") as pp1,
        ):
            # ---- persistent SBUF ----
            wx_t = [pers.tile([P, 2, N], BF16, tag=f"wx{c}", name=f"wx{c}") for c in range(NCHUNK)]
            W8_t = [pers.tile([P, 2, D], FP8, tag=f"W8{p}", name=f"W8{p}") for p in range(NPAIR)]
            xa_t = [pers.tile([P, 2, HEADS * HP], FP8, tag=f"xa{p}", name=f"xa{p}") for p in range(NPAIR)]
            qkT_t = [pers.tile([P, N], f32r, tag=f"qk{c}", name=f"qk{c}") for c in range(NCHUNK)]
            qn8_t = [pers.tile([32, 2, 2, N], FP8, tag=f"q8{c}", name=f"q8{c}") for c in range(NCHUNK)]
            att_t = [pers.tile([P, 2, N], FP8, tag=f"at{p}", name=f"at{p}") for p in range(NPAIR)]
            if gamma != 0.0:
                xT0_t = [pers.tile([P, N], BF16, tag=f"x0{c}", name=f"x0{c}") for c in range(NCHUNK)]
                ad_t = [pers.tile([P, N], f32r, tag=f"ad{c}", name=f"ad{c}") for c in range(NCHUNK)]
            else:
                ad_t = qkT_t

            bdc_t = small.tile([P, 2], BF16, tag="bdc")
            bd2_t = small.tile([2, P], BF16, tag="bd2")
            bmat_t = small.tile([P, NCHUNK], f32, tag="bmat")
            bmat2_t = small.tile([P, NCHUNK], f32, tag="bmat2")
            id_t = small.tile([P, P], f32r, tag="idn")

            # PE p-state warmup: ~5us of dependency-free matmuls during the
            # input-DMA wait ramps the tensor engine to full clock; the ramp
            # state persists across later idle gaps.
            warm_t = small.tile([P, FH], BF16, tag="warm")
            nc.gpsimd.memset(warm_t[:], 0.25)
            wps = pp1.tile([P, FH], f32, tag="p1", name="wps")
            for i in range(8):
                nc.tensor.matmul(wps[:], warm_t[:, 0:P], warm_t[:],
                                 start=True, stop=True)

            nc.sync.dma_start(bdc_t[:], bdc_d[:])
            nc.sync.dma_start(bmat_t[:], bmat_d[:])
            # combined W.T|x.T chunk loads: proj1(0)'s k-th matmul needs the
            # k-th chunk of both tensors; one DMA delivers the pair
            for c in range(NCHUNK):
                nc.sync.dma_start(wx_t[c][:], wx_d[c * P:(c + 1) * P, :, :])
            nc.sync.dma_start(bd2_t[:], bd2_d[:])
            for p in range(NPAIR):
                nc.sync.dma_start(W8_t[p][:], W8_d[p * P:(p + 1) * P, :, :])
                nc.sync.dma_start(xa_t[p][:], xa_d[p * P:(p + 1) * P, :, :])
            nc.sync.dma_start(bmat2_t[:], bmat2_d[:])
            nc.sync.dma_start(id_t[:], id_d[:])
            if gamma != 0.0:
                for c in range(NCHUNK):
                    nc.sync.dma_start(xT0_t[c][:], xT0_d[c * P:(c + 1) * P, :])

            def emit_proj1(c):
                # qk_c = (x @ W.T)_c + b_c
                for fn in range(2):
                    ps = pp1.tile([P, FH], f32, tag="p1", name="ps")
                    for k in range(NCHUNK):
                        nc.tensor.matmul(
                            ps[:],
                            wx_t[k][:, 0, c * P:(c + 1) * P],
                            wx_t[k][:, 1, fn * FH:(fn + 1) * FH],
                            start=(k == 0), stop=(k == NCHUNK - 1))
                    nc.vector.tensor_scalar_add(
                        qkT_t[c][:, fn * FH:(fn + 1) * FH], ps[:],
                        bmat_t[:, c:c + 1])
                if gamma != 0.0:
                    # separate x @ W.T for the final-output addend
                    for fn in range(2):
                        ps = pp1.tile([P, FH], f32, tag="p1", name="ps0")
                        for k in range(NCHUNK):
                            nc.tensor.matmul(
                                ps[:],
                                wx_t[k][:, 0, c * P:(c + 1) * P],
                                xT0_t[k][:, fn * FH:(fn + 1) * FH],
                                start=(k == 0), stop=(k == NCHUNK - 1))
                        nc.vector.tensor_scalar(
                            ad_t[c][:, fn * FH:(fn + 1) * FH], ps[:],
                            float(w1 / W0S), bmat2_t[:, c:c + 1], MULT, ADD)

            qn_stage = {}

            def emit_norm(c, psum_pool=None):
                # l2norm pair for heads (2c, 2c+1): invn = rsqrt(ssq) via
                # quake seed + one Newton step (DVE; Pool only does sq so the
                # Pool FIFO stays clear for the per-head Z broadcasts).
                # chunk 0 borrows the (still idle) gram psum pool and runs sq
                # on DVE so the startup chain never blocks proj1(1+) slots.
                pool = psum_pool if psum_pool is not None else pp1
                sq = sqp.tile([P, N], BF16, tag="sq", name="sq")
                if c == 0:
                    nc.vector.tensor_mul(sq[:], qkT_t[c][:], qkT_t[c][:])
                else:
                    nc.gpsimd.tensor_mul(sq[:], qkT_t[c][:], qkT_t[c][:])
                y0 = invp.tile([2, N], i32, tag="y0", name="y0")
                sd = invp.tile([2, N], f32, tag="sd", name="sd")
                t2 = invp.tile([2, N], f32, tag="t2", name="t2")
                inv = invp.tile([2, N], BF16, tag="inv", name="inv")
                ssqs = []
                for fn in range(2):
                    ssq = pool.tile([P, FH] if psum_pool is None else [P, N],
                                    f32, tag="p1" if psum_pool is None else "pg",
                                    name="ssq")
                    nc.tensor.matmul(
                        ssq[0:2, 0:FH], bdc_t[:], sq[:, fn * FH:(fn + 1) * FH],
                        start=True, stop=True)
                    ssqs.append(ssq)
                if c == 0:
                    # startup chunk: ACT is idle, so do rsqrt as
                    # exp(-0.5*ln(ssq)) there; both table loads hide in the
                    # pre-attention idle and the DVE chain shortens by ~8us
                    Ln = mybir.ActivationFunctionType.Ln
                    for fn in range(2):
                        nc.scalar.activation(
                            sd[:, fn * FH:(fn + 1) * FH], ssqs[fn][0:2, 0:FH],
                            Ln)
                    nc.scalar.activation(inv[:], sd[:], Exp, scale=-0.5)
                else:
                    for fn in range(2):
                        # PSUM reads convert (not reinterpret), so stage ssq
                        # in SBUF before the integer bit-trick reads below.
                        nc.vector.tensor_copy(
                            sd[:, fn * FH:(fn + 1) * FH], ssqs[fn][0:2, 0:FH])
                    # quake seed: y0 = QC - (ssq_bits >> 1), one Newton step
                    nc.vector.tensor_scalar(y0[:], sd[:].bitcast(i32), 1,
                                            None, LSR)
                    y0f = y0[:].bitcast(f32)
                    nc.vector.tensor_scalar(y0[:], y0[:], QC, -1, SUB, MULT)
                    nc.vector.tensor_mul(t2[:], y0f, y0f)
                    nc.vector.tensor_mul(sd[:], sd[:], t2[:])
                    # inv = y0 * (1.5 - 0.5*x*y0^2)
                    nc.vector.tensor_scalar(t2[:], sd[:], -0.5, 1.5, MULT, ADD)
                    nc.vector.tensor_mul(inv[:], t2[:], y0f)

                # qn = qk * bcast(8*invn) -> fp8 (bd2 holds 8.0s: fp8 scale)
                qn = qnp.tile([P, N], FP8, tag="qn", name="qn")
                for fn in range(2):
                    pbt = pool.tile([P, FH] if psum_pool is None else [P, N],
                                    f32, tag="p1" if psum_pool is None else "pg",
                                    name="pbt")
                    nc.tensor.matmul(
                        pbt[:, 0:FH], bd2_t[:], inv[:, fn * FH:(fn + 1) * FH],
                        start=True, stop=True)
                    nc.vector.tensor_mul(
                        qn[:, fn * FH:(fn + 1) * FH],
                        qkT_t[c][:, fn * FH:(fn + 1) * FH], pbt[:, 0:FH])
                qn_stage[c] = qn
                if c == 0:
                    return  # heads 0/1 read the staging tile directly
                # relayout each half into DoubleRow [32, 2, N] with one DMA:
                # flat AP pairing maps head-dim d = 2p+s, which is a valid
                # contraction order because the gram uses the same tile on
                # both sides
                for half in range(2):
                    nc.sync.dma_start(
                        qn8_t[c][:, half, :, :],
                        qn[half * HD:(half + 1) * HD, :])

            with (
                tc.tile_pool(name="E", bufs=7) as pE,
                tc.tile_pool(name="rzp", bufs=2) as rzp,
                tc.tile_pool(name="psum_g", bufs=2, space="PSUM") as pg_pool,
                tc.tile_pool(name="psum_av", bufs=1, space="PSUM") as pav_pool,
            ):
                last_E = []

                def emit_head(h):
                    c, half = h // 2, h % 2
                    pav = pav_pool.tile([HD + 1, 2, FH], f32, tag="pav", name="pav")
                    for p in range(NPAIR):
                        Ep = pE.tile([P, 2, N], FP8, tag="E", name="Ep")
                        if h == 2 * NCHUNK - 1:
                            last_E.append(Ep)
                        for s in range(2):
                            mb = 2 * p + s
                            pg = pg_pool.tile([P, N], f32, tag="pg", name="pg")
                            for fn in range(2):
                                if c == 0:
                                    # chunk 0: plain fp8 matmul off the qn
                                    # staging tile (no relayout wait)
                                    qs = qn_stage[0][half * HD:(half + 1) * HD, :]
                                    nc.tensor.matmul(
                                        pg[:, fn * FH:(fn + 1) * FH],
                                        qs[:, mb * P:(mb + 1) * P],
                                        qs[:, fn * FH:(fn + 1) * FH],
                                        start=True, stop=True)
                                else:
                                    qn_h = qn8_t[c][:, half, :, :]
                                    nc.tensor.matmul(
                                        pg[:, fn * FH:(fn + 1) * FH],
                                        qn_h[:, :, mb * P:(mb + 1) * P],
                                        qn_h[:, :, fn * FH:(fn + 1) * FH],
                                        start=True, stop=True, perf_mode=DR)
                            nc.scalar.activation(Ep[:, s, :], pg[:], Exp,
                                                 scale=logit_scale / 64.0)
                        for fn in range(2):
                            nc.tensor.matmul(
                                pav[:, fn, :],
                                xa_t[p][:, :, h * HP:h * HP + HD + 1],
                                Ep[:, :, fn * FH:(fn + 1) * FH],
                                start=(p == 0), stop=(p == NPAIR - 1),
                                perf_mode=DR)
                    # Z row must reach SBUF via a plain DVE copy: the custom
                    # reciprocal ucode (and all gpsimd ops) cannot read PSUM.
                    zr = rzp.tile([1, 2, FH], f32, tag="zr", name="zr")
                    nc.vector.tensor_copy(zr[:], pav[HD:HD + 1, :, :])
                    rz = rzp.tile([1, 2, FH], f32, tag="rz", name="rz")
                    nc.vector.reciprocal_approx_fast(rz[:], zr[:])
                    rzb = rzp.tile([P, N], f32, tag="rzb", name="rzb")
                    nc.gpsimd.partition_broadcast(rzb[:], rz[:])
                    ap, sp = h // 4, (h // 2) % 2
                    for fn in range(2):
                        nc.vector.tensor_mul(
                            att_t[ap][half * HD:(half + 1) * HD, sp,
                                      fn * FH:(fn + 1) * FH],
                            pav[0:HD, fn, :],
                            rzb[half * HD:(half + 1) * HD,
                                fn * FH:(fn + 1) * FH])

                # window pipeline: proj1 one chunk ahead; the norm chain for
                # chunk c+1 is emitted between the two heads of chunk c so
                # the Pool FIFO serves head (2c)'s broadcast before chunk
                # c+1's sq (avoids head-of-line blocking).
                def emit_addend(c):
                    # addend (prescaled by 1/w0s so the proj2 drain is a pure
                    # psum*w0s copy on ACT): ad = (w1*qk + (1-w1)*b) / w0s,
                    # in place over qk (Pool; emitted a window after its
                    # dependencies clear so it never stalls the Pool FIFO)
                    if gamma == 0.0:
                        nc.gpsimd.tensor_scalar(
                            qkT_t[c][:], qkT_t[c][:],
                            float(w1 / W0S), bmat2_t[:, c:c + 1], MULT, ADD)

                # software pipeline: proj1 two windows ahead, norm one ahead
                emit_proj1(0)
                emit_norm(0, pg_pool)
                emit_proj1(1)
                for c in range(NCHUNK):
                    if c + 2 < NCHUNK:
                        emit_proj1(c + 2)
                    emit_head(2 * c)
                    if c + 1 < NCHUNK:
                        emit_norm(c + 1)
                    if c >= 1:
                        emit_addend(c - 1)
                    emit_head(2 * c + 1)
                emit_addend(NCHUNK - 1)

                # keep the tensor engine clocked through the last head's
                # serial Z-normalization chain: these fillers depend on the
                # final E tile, so the in-order PE stream runs them in the
                # otherwise-idle gap right before the final projection
                wps2 = pp1.tile([P, FH], f32, tag="p1", name="wps2")
                for i in range(24):
                    nc.tensor.matmul(wps2[:], last_E[NPAIR - 1][:, 1, 0:P],
                                     warm_t[:], start=True, stop=True)

                if os.environ.get("BK_DEBUG", "0") == "1":
                    dbg = {
                        "d_qk0": qkT_t[0],
                        "d_qn8_1": qn8_t[1],
                        "d_att0": att_t[0],
                        "d_xa0": xa_t[0],
                        "d_w80": W8_t[0],
                    }
                    for nm, t in dbg.items():
                        sh = list(t[:].shape)
                        flat = [sh[0], int(np.prod(sh[1:]))]
                        dd = nc.declare_dram_parameter(nm, flat, t.tensor.dtype,
                                                       isOutput=True)
                        nc.sync.dma_start(dd[:], t[:])

                # ---- final projection: fin = w0s*psum, psum = att@W8 + ad
                # (reuses the gram psum pool -- no pool-close barrier -- and
                # alternates odd groups onto the p1 pool so three psum groups
                # stay in flight; drains alternate ACT/DVE)
                for m in range(NCHUNK):
                    fin = pfin.tile([P, N], BF16, tag="fin", name="fin")
                    if m % 2 == 0:
                        ps2f = [pg_pool.tile([P, N], f32, tag="pg", name="ps2")]
                        slc = [(ps2f[0], fn * FH) for fn in range(2)]
                    else:
                        a = pp1.tile([P, FH], f32, tag="p1", name="p2a")
                        bb = pp1.tile([P, FH], f32, tag="p1", name="p2b")
                        slc = [(a, 0), (bb, 0)]
                    for fn in range(2):
                        t, off = slc[fn]
                        # pairs 0-2 and the addend identity are ready well
                        # before the last head; only the pair-3 matmul (and
                        # the group stop) waits for heads 12-15
                        for p in range(NPAIR - 1):
                            nc.tensor.matmul(
                                t[:, off:off + FH],
                                W8_t[p][:, :, m * P:(m + 1) * P],
                                att_t[p][:, :, fn * FH:(fn + 1) * FH],
                                start=(p == 0), stop=False,
                                perf_mode=DR)
                        nc.tensor.matmul(
                            t[:, off:off + FH],
                            id_t[:],
                            ad_t[m][:, fn * FH:(fn + 1) * FH],
                            start=False, stop=False)
                        nc.tensor.matmul(
                            t[:, off:off + FH],
                            W8_t[NPAIR - 1][:, :, m * P:(m + 1) * P],
                            att_t[NPAIR - 1][:, :, fn * FH:(fn + 1) * FH],
                            start=False, stop=True,
                            perf_mode=DR)
                        if m % 2 == 1:
                            if fn == 0:
                                nc.scalar.mul(
                                    fin[:, 0:FH], t[:, 0:FH], float(W0S))
                            else:
                                nc.vector.tensor_scalar_mul(
                                    fin[:, FH:N], t[:, 0:FH], float(W0S))
                    if m % 2 == 0:
                        nc.vector.tensor_scalar_mul(
                            fin[:], slc[0][0][:], float(W0S))
                    nc.sync.dma_start(out_d[m * P:(m + 1) * P, :], fin[:])

    nc.compile()
    return nc


def _host_prep(x, pos, W, b, gamma, w0, w1):
    """Per-core input shards (host layout work only)."""
    import ml_dtypes

    bf16 = ml_dtypes.bfloat16
    f8 = ml_dtypes.float8_e4m3

    WT = np.ascontiguousarray(W.T)                        # [C, D] f32
    WTb = WT.astype(bf16)                                 # [C, D] bf16
    # pair layout: W8[pair*128 + part, s, d] = 32*W.T[128*(2*pair+s)+part, d]
    W8 = (W8S * WT).astype(f8).reshape(NPAIR, 2, P, D).transpose(0, 2, 1, 3)
    W8 = np.ascontiguousarray(W8).reshape(NPAIR * P, 2, D)
    bmat = np.ascontiguousarray(b.reshape(NCHUNK, P).T)   # [P, 8]
    w0s = w0 / (W8S * VSC)
    bmat2 = np.ascontiguousarray(
        ((b / w0s if gamma != 0.0 else (1.0 - w1) / w0s * b)).reshape(NCHUNK, P).T)
    idn = np.eye(P, dtype=np.float32)
    bdc = np.zeros((P, 2), dtype=bf16)
    bdc[:HD, 0] = 1.0
    bdc[HD:, 1] = 1.0
    # 8.0 (exact in bf16): scales qn into fp8-friendly range; compensated by
    # exp(scale=logit_scale/64)
    bd2 = np.zeros((2, P), dtype=bf16)
    bd2[0, :HD] = 8.0
    bd2[1, HD:] = 8.0

    in_maps = []
    for i in range(B):
        xi = x[i]                                         # [N, C]
        xa = np.zeros((N, HEADS, HP), dtype=np.float32)
        xa[:, :, :HD] = xi.reshape(N, HEADS, HD)
        xa[:, :, HD] = 1.0 / VSC
        xa8 = xa.astype(f8).reshape(NPAIR, 2, P, HEADS * HP)
        xa8 = np.ascontiguousarray(xa8.transpose(0, 2, 1, 3))
        m = {
            "W8": W8,
            "xa": xa8.reshape(NPAIR * P, 2, HEADS * HP),
            "bdc": bdc,
            "bd2": bd2,
            "idn": idn,
            "bmat": bmat,
            "bmat2": bmat2,
        }
        wx = np.empty((C, 2, N), dtype=bf16)
        wx[:, 0, :] = WTb
        if gamma != 0.0:
            xp = xi + gamma * pos[i].reshape(C, N).T
            wx[:, 1, :] = xp.T.astype(bf16)
            m["xT0"] = np.ascontiguousarray(xi.T).astype(bf16)
        else:
            wx[:, 1, :] = xi.T.astype(bf16)
        m["wx"] = wx
        in_maps.append(m)
    return in_maps


LAST_RESULT = None


def kernel(x, pos, W, b, gamma, attn_gamma, sum_gamma0, sum_gamma1):
    global LAST_RESULT
    import sys
    sys.path.insert(0, "/opt/trn_rl_repo")
    from concourse.bass_utils import run_bass_kernel_spmd

    x = np.asarray(x, dtype=np.float32)
    pos = np.asarray(pos, dtype=np.float32)
    W = np.asarray(W, dtype=np.float32)
    b = np.asarray(b, dtype=np.float32)
    gamma = float(np.asarray(gamma))
    attn_gamma = float(np.asarray(attn_gamma))
    g0 = math.exp(float(np.asarray(sum_gamma0)))
    g1 = math.exp(float(np.asarray(sum_gamma1)))
    w0, w1 = g0 / (g0 + g1), g1 / (g0 + g1)
    logit_scale = math.sqrt(HD) / attn_gamma

    nc = _build(gamma, w0, w1, logit_scale)
    in_maps = _host_prep(x, pos, W, b, gamma, w0, w1)
    res = run_bass_kernel_spmd(
        nc, in_maps, core_ids=list(range(B)),
        trace=os.environ.get("BK_TRACE", "0") == "1",
    )
    LAST_RESULT = res
    out = np.empty((B, N, D), dtype=np.float32)
    for i in range(B):
        out[i] = res.results[i]["out"].astype(np.float32).T
    return out


# revision 83
# speedup vs baseline: 1.0062x; 1.0008x over previous
"""Trainium2 Bass kernel for nn_Attention_27376121544790.

Math (per batch element, B=8 -> one element per NeuronCore, no collectives):
  qk   = x @ W.T + b                              [N, D] (on device: [D, N])
  q = k = l2norm(qk per 64-dim head)
  S    = (q @ k.T) * (sqrt(64)/attn_gamma)        per head
  attn = softmax(S) = E / Z,  E = exp(S), Z = col sums (E symmetric)
  out  = attn @ v,  v = x head-split
  final= w0*(out @ W.T) + w1*qk + (1-w1)*b        (uses x@W.T = qk - b, so the
         blend projection collapses into the already-computed qk)

Key engine assignments / formats (ACT exp of the N^2 logits is the
bottleneck engine; everything else is arranged to keep it fed):
  - proj1 (x@W.T) in bf16 (accuracy: qk feeds the final output directly),
    W.T and x.T interleaved in one "wx" tensor so the contraction loop can
    chase combined chunk-pair DMAs.
  - q/k (qn8), E, v (xaug), att and W8 in fp8e4 DoubleRow pair layouts
    ([Ki, 2, free]); the gram, attn@v and final att-projection all run as
    fp8 DoubleRow matmuls. att is scaled x32 into fp8 range via a 1/32
    ones column in xaug (Z comes out as Z/32; recip gives 32/Z); W8 is
    32*W; qn8 is 8*qn (compensated in the exp scale ls/64).
  - l2norm: per-chunk ssq via block-ones matmul; invn = rsqrt(ssq) via the
    quake bit-trick + one Newton step on DVE (SBUF staging first: PSUM
    reads convert, they do not reinterpret bits). Chunk 0 instead uses
    ACT Ln/Exp while ACT is still idle. Broadcast across partitions on
    gpsimd (which can never touch PSUM).
  - addend w1*qk+(1-w1)*b is pre-scaled by 1/w0s and folded into the
    proj2 accumulator through an identity matmul, so proj2 drains are
    pure scale-copies alternating ACT/DVE.
  - PE p-state: a dependency-free warmup block ramps the tensor engine
    to full clock before the first real matmul; the ramp persists.
  - Window pipeline: proj1 two chunks ahead, norm chain one ahead; heads
    (2c, 2c+1) start as soon as chunk c is normalized, overlapping the
    projection and norm work with the ACT-bound softmax stream.
"""

import math
import os

import numpy as np

B, N, C, D = 8, 1024, 1024, 1024
HEADS, HD = 16, 64
P = 128
EPS = 1e-6
NCHUNK = C // P      # 8 chunks of 128 feature rows
NPAIR = NCHUNK // 2  # 4 DoubleRow pair chunks
FH = 512             # free-dim half (one PSUM bank of f32)
HP = 80              # padded per-head stride in xaug (65 used, %16 == 0)
W8S = 32.0           # fp8 scale on W
VSC = 32.0           # att scale (via 1/32 ones column)


def _build(gamma: float, w0: float, w1: float, logit_scale: float):
    import concourse.bass as bass
    import concourse.tile as tile
    from concourse import bacc, mybir

    f32 = mybir.dt.float32
    f32r = mybir.dt.float32r
    BF16 = mybir.dt.bfloat16
    FP8 = mybir.dt.float8e4
    DR = mybir.MatmulPerfMode.DoubleRow

    Exp = mybir.ActivationFunctionType.Exp
    MULT = mybir.AluOpType.mult
    ADD = mybir.AluOpType.add
    SUB = mybir.AluOpType.subtract
    LSR = mybir.AluOpType.logical_shift_right
    i32 = mybir.dt.int32
    QC = 0x5F3759DF  # quake rsqrt seed constant

    W0S = w0 / (W8S * VSC)  # proj2 drain scale

    nc = bacc.Bacc("TRN2", target_bir_lowering=False, debug=False)

    wx_d = nc.declare_dram_parameter("wx", [C, 2, N], BF16, isOutput=False)
    if gamma != 0.0:
        xT0_d = nc.declare_dram_parameter("xT0", [C, N], BF16, isOutput=False)
    W8_d = nc.declare_dram_parameter("W8", [NPAIR * P, 2, D], FP8, isOutput=False)
    xa_d = nc.declare_dram_parameter("xa", [NPAIR * P, 2, HEADS * HP], FP8, isOutput=False)
    bdc_d = nc.declare_dram_parameter("bdc", [P, 2], BF16, isOutput=False)
    bd2_d = nc.declare_dram_parameter("bd2", [2, P], BF16, isOutput=False)
    id_d = nc.declare_dram_parameter("idn", [P, P], f32r, isOutput=False)
    bmat_d = nc.declare_dram_parameter("bmat", [P, NCHUNK], f32, isOutput=False)
    bmat2_d = nc.declare_dram_parameter("bmat2", [P, NCHUNK], f32, isOutput=False)
    out_d = nc.declare_dram_parameter("out", [D, N], BF16, isOutput=True)

    with tile.TileContext(nc) as tc:
        with (
            tc.tile_pool(name="pers", bufs=1) as pers,
            tc.tile_pool(name="small", bufs=1) as small,
            tc.tile_pool(name="sqp", bufs=2) as sqp,
            tc.tile_pool(name="invp", bufs=2) as invp,
            tc.tile_pool(name="qnp", bufs=2) as qnp,
            tc.tile_pool(name="fin", bufs=3) as pfin,
            tc.tile_pool(name="psum_p1", bufs=2, space="PSUM") as pp1,
        ):
            # ---- persistent SBUF ----
            wx_t = [pers.tile([P, 2, N], BF16, tag=f"wx{c}", name=f"wx{c}") for c in range(NCHUNK)]
            W8_t = [pers.tile([P, 2, D], FP8, tag=f"W8{p}", name=f"W8{p}") for p in range(NPAIR)]
            xa_t = [pers.tile([P, 2, HEADS * HP], FP8, tag=f"xa{p}", name=f"xa{p}") for p in range(NPAIR)]
            qkT_t = [pers.tile([P, N], f32r, tag=f"qk{c}", name=f"qk{c}") for c in range(NCHUNK)]
            qn8_t = [pers.tile([32, 2, 2, N], FP8, tag=f"q8{c}", name=f"q8{c}") for c in range(NCHUNK)]
            att_t = [pers.tile([P, 2, N], FP8, tag=f"at{p}", name=f"at{p}") for p in range(NPAIR)]
            if gamma != 0.0:
                xT0_t = [pers.tile([P, N], BF16, tag=f"x0{c}", name=f"x0{c}") for c in range(NCHUNK)]
                ad_t = [pers.tile([P, N], f32r, tag=f"ad{c}", name=f"ad{c}") for c in range(NCHUNK)]
            else:
                ad_t = qkT_t

            bdc_t = small.tile([P, 2], BF16, tag="bdc")
            bd2_t = small.tile([2, P], BF16, tag="bd2")
            bmat_t = small.tile([P, NCHUNK], f32, tag="bmat")
            bmat2_t = small.tile([P, NCHUNK], f32, tag="bmat2")
            id_t = small.tile([P, P], f32r, tag="idn")

            # PE p-state warmup: ~5us of dependency-free matmuls during the
            # input-DMA wait ramps the tensor engine to full clock; the ramp
            # state persists across later idle gaps.
            warm_t = small.tile([P, FH], BF16, tag="warm")
            nc.gpsimd.memset(warm_t[:], 0.25)
            wps = pp1.tile([P, FH], f32, tag="p1", name="wps")
            for i in range(8):
                nc.tensor.matmul(wps[:], warm_t[:, 0:P], warm_t[:],
                                 start=True, stop=True)

            nc.sync.dma_start(bdc_t[:], bdc_d[:])
            nc.sync.dma_start(bmat_t[:], bmat_d[:])
            # combined W.T|x.T chunk loads: proj1(0)'s k-th matmul needs the
            # k-th chunk of both tensors; one DMA delivers the pair
            for c in range(NCHUNK):
                nc.sync.dma_start(wx_t[c][:], wx_d[c * P:(c + 1) * P, :, :])
            nc.sync.dma_start(bd2_t[:], bd2_d[:])
            for p in range(NPAIR):
                nc.sync.dma_start(W8_t[p][:], W8_d[p * P:(p + 1) * P, :, :])
                nc.sync.dma_start(xa_t[p][:], xa_d[p * P:(p + 1) * P, :, :])
            nc.sync.dma_start(bmat2_t[:], bmat2_d[:])
            nc.sync.dma_start(id_t[:], id_d[:])
            if gamma != 0.0:
                for c in range(NCHUNK):
                    nc.sync.dma_start(xT0_t[c][:], xT0_d[c * P:(c + 1) * P, :])

            def emit_proj1(c):
                # qk_c = (x @ W.T)_c + b_c
                for fn in range(2):
                    ps = pp1.tile([P, FH], f32, tag="p1", name="ps")
                    for k in range(NCHUNK):
                        nc.tensor.matmul(
                            ps[:],
                            wx_t[k][:, 0, c * P:(c + 1) * P],
                            wx_t[k][:, 1, fn * FH:(fn + 1) * FH],
                            start=(k == 0), stop=(k == NCHUNK - 1))
                    nc.vector.tensor_scalar_add(
                        qkT_t[c][:, fn * FH:(fn + 1) * FH], ps[:],
                        bmat_t[:, c:c + 1])
                if gamma != 0.0:
                    # separate x @ W.T for the final-output addend
                    for fn in range(2):
                        ps = pp1.tile([P, FH], f32, tag="p1", name="ps0")
                        for k in range(NCHUNK):
                            nc.tensor.matmul(
                                ps[:],
                                wx_t[k][:, 0, c * P:(c + 1) * P],
                                xT0_t[k][:, fn * FH:(fn + 1) * FH],
                                start=(k == 0), stop=(k == NCHUNK - 1))
                        nc.vector.tensor_scalar(
                            ad_t[c][:, fn * FH:(fn + 1) * FH], ps[:],
                            float(w1 / W0S), bmat2_t[:, c:c + 1], MULT, ADD)

            qn_stage = {}

            def emit_norm(c, psum_pool=None):
                # l2norm pair for heads (2c, 2c+1): invn = rsqrt(ssq) via
                # quake seed + one Newton step (DVE; Pool only does sq so the
                # Pool FIFO stays clear for the per-head Z broadcasts).
                # chunk 0 borrows the (still idle) gram psum pool and runs sq
                # on DVE so the startup chain never blocks proj1(1+) slots.
                pool = psum_pool if psum_pool is not None else pp1
                sq = sqp.tile([P, N], BF16, tag="sq", name="sq")
                if c == 0:
                    nc.vector.tensor_mul(sq[:], qkT_t[c][:], qkT_t[c][:])
                else:
                    nc.gpsimd.tensor_mul(sq[:], qkT_t[c][:], qkT_t[c][:])
                y0 = invp.tile([2, N], i32, tag="y0", name="y0")
                sd = invp.tile([2, N], f32, tag="sd", name="sd")
                t2 = invp.tile([2, N], f32, tag="t2", name="t2")
                inv = invp.tile([2, N], BF16, tag="inv", name="inv")
                ssqs = []
                for fn in range(2):
                    ssq = pool.tile([P, FH] if psum_pool is None else [P, N],
                                    f32, tag="p1" if psum_pool is None else "pg",
                                    name="ssq")
                    nc.tensor.matmul(
                        ssq[0:2, 0:FH], bdc_t[:], sq[:, fn * FH:(fn + 1) * FH],
                        start=True, stop=True)
                    ssqs.append(ssq)
                if c == 0:
                    # startup chunk: ACT is idle, so do rsqrt as
                    # exp(-0.5*ln(ssq)) there; both table loads hide in the
                    # pre-attention idle and the DVE chain shortens by ~8us
                    Ln = mybir.ActivationFunctionType.Ln
                    for fn in range(2):
                        nc.scalar.activation(
                            sd[:, fn * FH:(fn + 1) * FH], ssqs[fn][0:2, 0:FH],
                            Ln)
                    nc.scalar.activation(inv[:], sd[:], Exp, scale=-0.5)
                else:
                    for fn in range(2):
                        # PSUM reads convert (not reinterpret), so stage ssq
                        # in SBUF before the integer bit-trick reads below.
                        nc.vector.tensor_copy(
                            sd[:, fn * FH:(fn + 1) * FH], ssqs[fn][0:2, 0:FH])
                    # quake seed: y0 = QC - (ssq_bits >> 1), one Newton step
                    nc.vector.tensor_scalar(y0[:], sd[:].bitcast(i32), 1,
                                            None, LSR)
                    y0f = y0[:].bitcast(f32)
                    nc.vector.tensor_scalar(y0[:], y0[:], QC, -1, SUB, MULT)
                    nc.vector.tensor_mul(t2[:], y0f, y0f)
                    nc.vector.tensor_mul(sd[:], sd[:], t2[:])
                    # inv = y0 * (1.5 - 0.5*x*y0^2)
                    nc.vector.tensor_scalar(t2[:], sd[:], -0.5, 1.5, MULT, ADD)
                    nc.vector.tensor_mul(inv[:], t2[:], y0f)

                # qn = qk * bcast(8*invn) -> fp8 (bd2 holds 8.0s: fp8 scale)
                qn = qnp.tile([P, N], FP8, tag="qn", name="qn")
                for fn in range(2):
                    pbt = pool.tile([P, FH] if psum_pool is None else [P, N],
                                    f32, tag="p1" if psum_pool is None else "pg",
                                    name="pbt")
                    nc.tensor.matmul(
                        pbt[:, 0:FH], bd2_t[:], inv[:, fn * FH:(fn + 1) * FH],
                        start=True, stop=True)
                    nc.vector.tensor_mul(
                        qn[:, fn * FH:(fn + 1) * FH],
                        qkT_t[c][:, fn * FH:(fn + 1) * FH], pbt[:, 0:FH])
                qn_stage[c] = qn
                if c == 0:
                    return  # heads 0/1 read the staging tile directly
                # relayout each half into DoubleRow [32, 2, N] with one DMA:
                # flat AP pairing maps head-dim d = 2p+s, which is a valid
                # contraction order because the gram uses the same tile on
                # both sides
                for half in range(2):
                    nc.sync.dma_start(
                        qn8_t[c][:, half, :, :],
                        qn[half * HD:(half + 1) * HD, :])

            with (
                tc.tile_pool(name="E", bufs=7) as pE,
                tc.tile_pool(name="rzp", bufs=2) as rzp,
                tc.tile_pool(name="psum_g", bufs=2, space="PSUM") as pg_pool,
                tc.tile_pool(name="psum_av", bufs=1, space="PSUM") as pav_pool,
            ):
                last_E = []

                def emit_head(h):
                    c, half = h // 2, h % 2
                    pav = pav_pool.tile([HD + 1, 2, FH], f32, tag="pav", name="pav")
                    for p in range(NPAIR):
                        Ep = pE.tile([P, 2, N], FP8, tag="E", name="Ep")
                        if h == 2 * NCHUNK - 1:
                            last_E.append(Ep)
                        for s in range(2):
                            mb = 2 * p + s
                            pg = pg_pool.tile([P, N], f32, tag="pg", name="pg")
                            for fn in range(2):
                                if c == 0:
                                    # chunk 0: plain fp8 matmul off the qn
                                    # staging tile (no relayout wait)
                                    qs = qn_stage[0][half * HD:(half + 1) * HD, :]
                                    nc.tensor.matmul(
                                        pg[:, fn * FH:(fn + 1) * FH],
                                        qs[:, mb * P:(mb + 1) * P],
                                        qs[:, fn * FH:(fn + 1) * FH],
                                        start=True, stop=True)
                                else:
                                    qn_h = qn8_t[c][:, half, :, :]
                                    nc.tensor.matmul(
                                        pg[:, fn * FH:(fn + 1) * FH],
                                        qn_h[:, :, mb * P:(mb + 1) * P],
                                        qn_h[:, :, fn * FH:(fn + 1) * FH],
                                        start=True, stop=True, perf_mode=DR)
                            nc.scalar.activation(Ep[:, s, :], pg[:], Exp,
                                                 scale=logit_scale / 64.0)
                        for fn in range(2):
                            nc.tensor.matmul(
                                pav[:, fn, :],
                                xa_t[p][:, :, h * HP:h * HP + HD + 1],
                                Ep[:, :, fn * FH:(fn + 1) * FH],
                                start=(p == 0), stop=(p == NPAIR - 1),
                                perf_mode=DR)
                    # Z row must reach SBUF via a plain DVE copy: the custom
                    # reciprocal ucode (and all gpsimd ops) cannot read PSUM.
                    zr = rzp.tile([1, 2, FH], f32, tag="zr", name="zr")
                    nc.vector.tensor_copy(zr[:], pav[HD:HD + 1, :, :])
                    rz = rzp.tile([1, 2, FH], f32, tag="rz", name="rz")
                    nc.vector.reciprocal_approx_fast(rz[:], zr[:])
                    rzb = rzp.tile([P, N], f32, tag="rzb", name="rzb")
                    nc.gpsimd.partition_broadcast(rzb[:], rz[:])
                    ap, sp = h // 4, (h // 2) % 2
                    for fn in range(2):
                        nc.vector.tensor_mul(
                            att_t[ap][half * HD:(half + 1) * HD, sp,
                                      fn * FH:(fn + 1) * FH],
                            pav[0:HD, fn, :],
                            rzb[half * HD:(half + 1) * HD,
                                fn * FH:(fn + 1) * FH])

                # window pipeline: proj1 one chunk ahead; the norm chain for
                # chunk c+1 is emitted between the two heads of chunk c so
                # the Pool FIFO serves head (2c)'s broadcast before chunk
                # c+1's sq (avoids head-of-line blocking).
                def emit_addend(c):
                    # addend (prescaled by 1/w0s so the proj2 drain is a pure
                    # psum*w0s copy on ACT): ad = (w1*qk + (1-w1)*b) / w0s,
                    # in place over qk (Pool; emitted a window after its
                    # dependencies clear so it never stalls the Pool FIFO)
                    if gamma == 0.0:
                        nc.gpsimd.tensor_scalar(
                            qkT_t[c][:], qkT_t[c][:],
                            float(w1 / W0S), bmat2_t[:, c:c + 1], MULT, ADD)

                # software pipeline: proj1 two windows ahead, norm one ahead
                emit_proj1(0)
                emit_norm(0, pg_pool)
                emit_proj1(1)
                for c in range(NCHUNK):
                    if c + 2 < NCHUNK:
                        emit_proj1(c + 2)
                    emit_head(2 * c)
                    if c + 1 < NCHUNK:
                        emit_norm(c + 1)
                    if c >= 1:
                        emit_addend(c - 1)
                    emit_head(2 * c + 1)
                emit_addend(NCHUNK - 1)

                # keep the tensor engine clocked through the last head's
                # serial Z-normalization chain: these fillers depend on the
                # final E tile, so the in-order PE stream runs them in the
                # otherwise-idle gap right before the final projection
                wps2 = pp1.tile([P, FH], f32, tag="p1", name="wps2")
                for i in range(16):
                    nc.tensor.matmul(wps2[:], last_E[NPAIR - 1][:, 1, 0:P],
                                     warm_t[:], start=True, stop=True)

                if os.environ.get("BK_DEBUG", "0") == "1":
                    dbg = {
                        "d_qk0": qkT_t[0],
                        "d_qn8_1": qn8_t[1],
                        "d_att0": att_t[0],
                        "d_xa0": xa_t[0],
                        "d_w80": W8_t[0],
                    }
                    for nm, t in dbg.items():
                        sh = list(t[:].shape)
                        flat = [sh[0], int(np.prod(sh[1:]))]
                        dd = nc.declare_dram_parameter(nm, flat, t.tensor.dtype,
                                                       isOutput=True)
                        nc.sync.dma_start(dd[:], t[:])

                # ---- final projection: fin = w0s*psum, psum = att@W8 + ad
                # (reuses the gram psum pool -- no pool-close barrier -- and
                # alternates odd groups onto the p1 pool so three psum groups
                # stay in flight; drains alternate ACT/DVE)
                for m in range(NCHUNK):
                    fin = pfin.tile([P, N], BF16, tag="fin", name="fin")
                    if m % 2 == 0:
                        ps2f = [pg_pool.tile([P, N], f32, tag="pg", name="ps2")]
                        slc = [(ps2f[0], fn * FH) for fn in range(2)]
                    else:
                        a = pp1.tile([P, FH], f32, tag="p1", name="p2a")
                        bb = pp1.tile([P, FH], f32, tag="p1", name="p2b")
                        slc = [(a, 0), (bb, 0)]
                    for fn in range(2):
                        t, off = slc[fn]
                        # pairs 0-2 and the addend identity are ready well
                        # before the last head; only the pair-3 matmul (and
                        # the group stop) waits for heads 12-15
                        for p in range(NPAIR - 1):
                            nc.tensor.matmul(
                                t[:, off:off + FH],
                                W8_t[p][:, :, m * P:(m + 1) * P],
                                att_t[p][:, :, fn * FH:(fn + 1) * FH],
                                start=(p == 0), stop=False,
                                perf_mode=DR)
                        nc.tensor.matmul(
                            t[:, off:off + FH],
                            id_t[:],
                            ad_t[m][:, fn * FH:(fn + 1) * FH],
                            start=False, stop=False)
                        nc.tensor.matmul(
                            t[:, off:off + FH],
                            W8_t[NPAIR - 1][:, :, m * P:(m + 1) * P],
                            att_t[NPAIR - 1][:, :, fn * FH:(fn + 1) * FH],
                            start=False, stop=True,
                            perf_mode=DR)
                        if m % 2 == 1:
                            if fn == 0:
                                nc.scalar.mul(
                                    fin[:, 0:FH], t[:, 0:FH], float(W0S))
                            else:
                                nc.vector.tensor_scalar_mul(
                                    fin[:, FH:N], t[:, 0:FH], float(W0S))
                    if m % 2 == 0:
                        nc.vector.tensor_scalar_mul(
                            fin[:], slc[0][0][:], float(W0S))
                    nc.sync.dma_start(out_d[m * P:(m + 1) * P, :], fin[:])

    nc.compile()
    return nc


def _host_prep(x, pos, W, b, gamma, w0, w1):
    """Per-core input shards (host layout work only)."""
    import ml_dtypes

    bf16 = ml_dtypes.bfloat16
    f8 = ml_dtypes.float8_e4m3

    WT = np.ascontiguousarray(W.T)                        # [C, D] f32
    WTb = WT.astype(bf16)                                 # [C, D] bf16
    # pair layout: W8[pair*128 + part, s, d] = 32*W.T[128*(2*pair+s)+part, d]
    W8 = (W8S * WT).astype(f8).reshape(NPAIR, 2, P, D).transpose(0, 2, 1, 3)
    W8 = np.ascontiguousarray(W8).reshape(NPAIR * P, 2, D)
    bmat = np.ascontiguousarray(b.reshape(NCHUNK, P).T)   # [P, 8]
    w0s = w0 / (W8S * VSC)
    bmat2 = np.ascontiguousarray(
        ((b / w0s if gamma != 0.0 else (1.0 - w1) / w0s * b)).reshape(NCHUNK, P).T)
    idn = np.eye(P, dtype=np.float32)
    bdc = np.zeros((P, 2), dtype=bf16)
    bdc[:HD, 0] = 1.0
    bdc[HD:, 1] = 1.0
    # 8.0 (exact in bf16): scales qn into fp8-friendly range; compensated by
    # exp(scale=logit_scale/64)
    bd2 = np.zeros((2, P), dtype=bf16)
    bd2[0, :HD] = 8.0
    bd2[1, HD:] = 8.0

    in_maps = []
    for i in range(B):
        xi = x[i]                                         # [N, C]
        xa = np.zeros((N, HEADS, HP), dtype=np.float32)
        xa[:, :, :HD] = xi.reshape(N, HEADS, HD)
        xa[:, :, HD] = 1.0 / VSC
        xa8 = xa.astype(f8).reshape(NPAIR, 2, P, HEADS * HP)
        xa8 = np.ascontiguousarray(xa8.transpose(0, 2, 1, 3))
        m = {
            "W8": W8,
            "xa": xa8.reshape(NPAIR * P, 2, HEADS * HP),
            "bdc": bdc,
            "bd2": bd2,
            "idn": idn,
            "bmat": bmat,
            "bmat2": bmat2,
        }
        wx = np.empty((C, 2, N), dtype=bf16)
        wx[:, 0, :] = WTb
        if gamma != 0.0:
            xp = xi + gamma * pos[i].reshape(C, N).T
            wx[:, 1, :] = xp.T.astype(bf16)
            m["xT0"] = np.ascontiguousarray(xi.T).astype(bf16)
        else:
            wx[:, 1, :] = xi.T.astype(bf16)
        m["wx"] = wx
        in_maps.append(m)
    return in_maps


LAST_RESULT = None


def kernel(x, pos, W, b, gamma, attn_gamma, sum_gamma0, sum_gamma1):
    global LAST_RESULT
    import sys
    sys.path.insert(0, "/opt/trn_rl_repo")
    from concourse.bass_utils import run_bass_kernel_spmd

    x = np.asarray(x, dtype=np.float32)
    pos = np.asarray(pos, dtype=np.float32)
    W = np.asarray(W, dtype=np.float32)
    b = np.asarray(b, dtype=np.float32)
    gamma = float(np.asarray(gamma))
    attn_gamma = float(np.asarray(attn_gamma))
    g0 = math.exp(float(np.asarray(sum_gamma0)))
    g1 = math.exp(float(np.asarray(sum_gamma1)))
    w0, w1 = g0 / (g0 + g1), g1 / (g0 + g1)
    logit_scale = math.sqrt(HD) / attn_gamma

    nc = _build(gamma, w0, w1, logit_scale)
    in_maps = _host_prep(x, pos, W, b, gamma, w0, w1)
    res = run_bass_kernel_spmd(
        nc, in_maps, core_ids=list(range(B)),
        trace=os.environ.get("BK_TRACE", "0") == "1",
    )
    LAST_RESULT = res
    out = np.empty((B, N, D), dtype=np.float32)
    for i in range(B):
        out[i] = res.results[i]["out"].astype(np.float32).T
    return out
